# revision 7
# baseline (speedup 1.0000x reference)
"""Trainium2 Bass kernel for nn_CompressSensory (embedding_lookup):
out = twohot_table[argmax(x, axis=1)] for x [1048576, 45] f32.

Pure data parallel over 8 NeuronCores, streamed in 8 chunks. End-to-end
wall time is dominated by the host<->device tunnel (~85 MB/s), so the
host sends a monotone 8-bit quantization of x (values below T=1.25
clamp to key 0 -- they essentially never win the argmax of 45 N(0,1)
samples) and the device returns the two-hot pattern bit-packed into a
u16 per row:

  - device (per row): group maxes over the 9 triangular spans of the
    two-hot table, offset-max chain, row max, equality one-hots -> the
    10 two-hot bits, dot with 2^c -> u16 code. (Same DVE pipeline as
    the exact-f32 kernel; quantization only changes the input dtype.)
  - host: codes whose bit pattern equals a row of twohot_table with a
    unique quantized max (always exactly-2-bit patterns; quantized
    ties always set >2 bits) decode by LUT; ambiguous rows (~2%) are
    recomputed exactly from the host-resident f32 x. Monotonicity of
    the quantizer makes every unique-key-max row's argmax exact, so
    the result is bit-exact with the reference.

Chunked device_put (async wire) overlaps host quantization with the
tunnel transfer; the compiled executable, transfer programs, and
device-resident dummy output buffers are built once per process and
cached. The execution path is the same PJRT custom-call lowering that
concourse.bass_utils.run_bass_kernel_spmd uses under axon, with the
jitted executable cached across calls instead of being rebuilt.
"""

import os
from concurrent.futures import ThreadPoolExecutor

import numpy as np

# Whole-tile dep granularity keeps per-instruction sync-wait counts low
# (walrus rejects DMA pseudo-instructions with >1 sync wait).
os.environ.setdefault("BY_DEFAULT_DISABLE_SUBTILE_DEPS", "1")

import jax
from jax.sharding import Mesh, PartitionSpec, NamedSharding
from jax.experimental.shard_map import shard_map

import concourse.bacc as bacc
import concourse.mybir as mybir
from concourse.tile import TileContext
from concourse import bass2jax

F32 = mybir.dt.float32
U8 = mybir.dt.uint8
U16 = mybir.dt.uint16

N_CORES = 8
ROWS_TOTAL = 1048576
X_DIM = 45
OUT_DIM = 10

CHUNKS = int(os.environ.get("K_CHUNKS", "8"))
CHUNK_ROWS = ROWS_TOTAL // CHUNKS          # rows per pipelined chunk
CORE_ROWS = CHUNK_ROWS // N_CORES          # rows per core per chunk
P = 128                                    # SBUF partitions
R = CORE_ROWS // P                         # rows per partition
QBLK = 16384                               # quantize row-block (cache-sized)

# Monotone u8 quantization: key = floor((clip(v, T, HI) - T) * S).
# Values < T (89% of N(0,1)) map to 0; rows whose max quantizes
# non-uniquely are detected on device and fixed exactly on host.
QT = 1.25
QHI = 6.0
QS = 255.0 / (QHI - QT)

TRI = [g * (g - 1) // 2 for g in range(1, 11)]

_CACHE = {}


def _build_nc():
    # Bacc (not bare Bass): finalize() runs generate_event_semaphores, which
    # splits multi-wait DMAs into event-semaphore + 1-wait DMA pairs.
    nc = bacc.Bacc()
    x_d = nc.declare_dram_parameter("xq", [CORE_ROWS, X_DIM], U8, isOutput=False)
    o_d = nc.declare_dram_parameter("oc", [CORE_ROWS], U16, isOutput=True)

    x_v = x_d.rearrange("(p r) d -> p (r d)", p=P, r=R)
    o_v = o_d.rearrange("(p r) -> p r", p=P, r=R)

    with TileContext(nc) as tc:
        with tc.tile_pool(name="pool", bufs=1) as pool:
            wb = pool.tile([P, OUT_DIM], F32, tag="wb")
            for c in range(OUT_DIM):
                nc.vector.memset(wb[:, c:c + 1], float(1 << c))

            xq = pool.tile([P, R * X_DIM], U8, tag="xq")
            nc.sync.dma_start(xq[:], x_v)
            xf = pool.tile([P, R * X_DIM], F32, tag="xf")
            nc.vector.tensor_copy(xf[:], xq[:])
            x3 = xf.rearrange("p (r d) -> p r d", d=X_DIM)

            # group maxes M_g (slot g-1), g=1..9
            Mst = pool.tile([P, R * 9], F32, tag="Mst")
            M3 = Mst.rearrange("p (r g) -> p r g", g=9)
            for g in range(1, 10):
                nc.vector.tensor_reduce(
                    M3[:, :, g - 1], x3[:, :, TRI[g - 1]:TRI[g]],
                    axis=mybir.AxisListType.X, op=mybir.AluOpType.max,
                )

            # offset-max chain, init folded: acc[0:8]=max(grp9[0:8],grp8)
            acc = pool.tile([P, R * 9], F32, tag="acc")
            a3 = acc.rearrange("p (r g) -> p r g", g=9)
            nc.vector.tensor_tensor(
                a3[:, :, 0:8], x3[:, :, TRI[8]:TRI[8] + 8],
                x3[:, :, TRI[7]:TRI[8]], mybir.AluOpType.max,
            )
            nc.vector.tensor_copy(a3[:, :, 8:9], x3[:, :, TRI[8] + 8:TRI[9]])
            for g in range(7, 0, -1):
                nc.vector.tensor_tensor(
                    a3[:, :, 0:g], a3[:, :, 0:g],
                    x3[:, :, TRI[g - 1]:TRI[g]], mybir.AluOpType.max,
                )

            mrow = pool.tile([P, R], F32, tag="mrow")
            nc.vector.tensor_reduce(
                mrow[:], a3, axis=mybir.AxisListType.X,
                op=mybir.AluOpType.max,
            )
            m_b9 = mrow.unsqueeze(2).broadcast_to([P, R, 9])
            m_b1 = mrow.unsqueeze(2).broadcast_to([P, R, 1])

            bt = pool.tile([P, R * OUT_DIM], F32, tag="bt")
            b3 = bt.rearrange("p (r e) -> p r e", e=OUT_DIM)

            # V-merge: slot k (k=1..8) feeds out col 9-k, needs group k
            # (M3 slot k-1): acc[1:9] = max(acc[1:9], M3[0:8]) in place
            nc.vector.tensor_tensor(
                a3[:, :, 1:9], a3[:, :, 1:9], M3[:, :, 0:8],
                mybir.AluOpType.max,
            )
            # cols 9..1 <- eq(acc[0:9], m) (reversed out AP)
            nc.vector.tensor_tensor(
                b3[:, :, 1:10][:, :, ::-1], a3, m_b9,
                mybir.AluOpType.is_equal,
            )
            # col 0 <- eq(M_9, m)
            nc.vector.tensor_tensor(
                b3[:, :, 0:1], M3[:, :, 8:9], m_b1,
                mybir.AluOpType.is_equal,
            )

            # pack bits: code = sum_c bit_c * 2^c, as u16
            nc.vector.tensor_tensor(
                b3, b3, wb.unsqueeze(1).broadcast_to([P, R, OUT_DIM]),
                mybir.AluOpType.mult,
            )
            cf = pool.tile([P, R], F32, tag="cf")
            nc.vector.tensor_reduce(
                cf[:], b3, axis=mybir.AxisListType.X, op=mybir.AluOpType.add,
            )
            c16 = pool.tile([P, R], U16, tag="c16")
            nc.vector.tensor_copy(c16[:], cf[:])
            nc.sync.dma_start(o_v, c16[:])
    return nc


def _get_rt():
    if "rt" in _CACHE:
        return _CACHE["rt"]
    bass2jax.install_neuronx_cc_hook()
    nc = _build_nc()
    if not nc.is_finalized():
        nc.finalize()

    # Mirror bass2jax.run_bass_via_pjrt's multi-core lowering exactly,
    # but build + jit the executable once and keep it cached.
    partition_name = nc.partition_id_tensor.name if nc.partition_id_tensor else None
    in_names, out_names, out_avals = [], [], []
    for alloc in nc.m.functions[0].allocations:
        if not isinstance(alloc, mybir.MemoryLocationSet):
            continue
        name = alloc.memorylocations[0].name
        if alloc.kind == "ExternalInput":
            if name != partition_name:
                in_names.append(name)
        elif alloc.kind == "ExternalOutput":
            out_names.append(name)
            out_avals.append(jax.core.ShapedArray(
                tuple(alloc.tensor_shape), mybir.dt.np(alloc.dtype)))
    all_names = list(in_names) + list(out_names)
    if partition_name is not None:
        all_names.append(partition_name)
    n_in = len(in_names) + len(out_names)

    def _body(*args):
        operands = list(args)
        if partition_name is not None:
            operands.append(bass2jax.partition_id_tensor())
        outs = bass2jax._bass_exec_p.bind(
            *operands,
            out_avals=tuple(out_avals),
            in_names=tuple(all_names),
            out_names=tuple(out_names),
            lowering_input_output_aliases=(),
            sim_require_finite=True,
            sim_require_nnan=True,
            nc=nc,
        )
        return tuple(outs)

    devices = jax.devices()[:N_CORES]
    mesh = Mesh(np.asarray(devices), ("core",))
    sh = NamedSharding(mesh, PartitionSpec("core"))
    fn = jax.jit(
        shard_map(
            _body, mesh=mesh,
            in_specs=(PartitionSpec("core"),) * n_in,
            out_specs=(PartitionSpec("core"),) * len(out_names),
            check_rep=False,
        ),
        keep_unused=True,
    )
    dummy_out = jax.device_put(np.zeros(CHUNK_ROWS, np.uint16), sh)

    # Warm everything once: H2D transfer program for the chunk shape,
    # NEFF compile + exec, D2H for the code shape.
    warm_in = jax.device_put(np.zeros((CHUNK_ROWS, X_DIM), np.uint8), sh)
    np.asarray(fn(warm_in, dummy_out)[0])
    del warm_in

    rt = {"fn": fn, "sh": sh, "dummy_out": dummy_out}
    _CACHE["rt"] = rt
    return rt


def _quantize_chunk(xs, tmp, out):
    """out = floor((clip(xs, QT, QHI) - QT) * QS) as u8, block-wise."""
    n = xs.shape[0]
    for lo in range(0, n, QBLK):
        hi = min(lo + QBLK, n)
        t = tmp[lo:hi]
        np.subtract(xs[lo:hi], QT, out=t)
        np.maximum(t, 0.0, out=t)
        np.minimum(t, QHI - QT, out=t)
        np.multiply(t, QS, out=t)
        out[lo:hi] = t  # float -> u8 truncation (monotone for v >= 0)


def _decode_lut(table):
    """code (10-bit) -> two-hot f32 row of `table`, + validity mask.

    A code is valid iff its bit pattern equals some row of the runtime
    table (always an exactly-2-bit pattern). Anything else -- quantized
    ties (>2 bits), all-below-threshold rows, unexpected patterns --
    is flagged for exact host fixup, so correctness never depends on
    the hardcoded triangular structure.
    """
    bits = ((np.arange(1024)[:, None] >> np.arange(OUT_DIM)) & 1)
    tcodes = (table.astype(np.int64) << np.arange(OUT_DIM)).sum(1).astype(np.int64)
    valid = np.zeros(1024, bool)
    valid[tcodes] = True
    lut = np.zeros((1024, OUT_DIM), np.float32)
    lut[tcodes] = table
    return lut, valid


def kernel(x, twohot_table):
    x = np.asarray(x)
    table = np.asarray(twohot_table, dtype=np.float32)
    assert x.shape == (ROWS_TOTAL, X_DIM) and x.dtype == np.float32, (
        x.shape, x.dtype)
    if not x.flags.c_contiguous:
        x = np.ascontiguousarray(x)

    rt = _get_rt()
    fn, sh, dummy_out = rt["fn"], rt["sh"], rt["dummy_out"]

    if "tmp" not in _CACHE:
        _CACHE["tmp"] = np.empty((CHUNK_ROWS, X_DIM), np.float32)
        _CACHE["ex"] = ThreadPoolExecutor(CHUNKS)
    tmp, ex = _CACHE["tmp"], _CACHE["ex"]

    lut, valid = _decode_lut(table)
    out = np.empty((ROWS_TOTAL, OUT_DIM), np.float32)

    def fetch_decode(h, lo):
        # np.asarray on the jax array releases the GIL until the codes
        # arrive, so these workers overlap the wire stream and hide the
        # D2H round-trip; the np.take decode is ~10ms per chunk.
        codes = np.asarray(h)
        np.take(lut, codes, axis=0, out=out[lo:lo + CHUNK_ROWS])
        return np.flatnonzero(~valid[codes]) + lo

    # Pipeline: quantize chunk k+1 on host while chunk k streams over
    # the wire; execs, D2H and decode overlap the stream per chunk.
    futs = []
    for k in range(CHUNKS):
        xs = x[k * CHUNK_ROWS:(k + 1) * CHUNK_ROWS]
        kq = np.empty((CHUNK_ROWS, X_DIM), np.uint8)
        _quantize_chunk(xs, tmp, kq)
        dk = jax.device_put(kq, sh)
        h = fn(dk, dummy_out)[0]
        futs.append(ex.submit(fetch_decode, h, k * CHUNK_ROWS))

    # Exact fixup from the host-resident f32 x (ambiguous ~2% of rows).
    bad_idx = [f.result() for f in futs]
    idx = np.concatenate(bad_idx)
    if idx.size:
        out[idx] = table[np.argmax(x[idx], axis=1)]
    return out
